# revision 70
# baseline (speedup 1.0000x reference)
"""Trainium2 Bass kernel for nn_LinearSelfAttention (B=8, N=4096, D=512).

Reference computation (per batch b):
    q = (phi @ Wq.T + bq) / sqrt(D)
    k =  phi @ Wk.T + bk
    v = weights[:, None] * (phi @ Wv.T + bv)
    phases = coords @ Wrot.T                # [N, D/2]
    q, k = rotary(q, phases), rotary(k, phases)
    out = q @ (k.T @ v)                     # linear attention, O(N*d^2)

Sharding: data-parallel over batch - batch element b runs on NeuronCore b
(8 cores, no collectives).

v30 design (v8 baseline ~184us on the tuning machine -> v30 ~137.3us;
528 matmuls x 216ns warm-clock stream = 114us is the PE floor):
  - Algebra as v8: the V projection is eliminated via
    MT[din,d] = sum_n (w_n phi[n,din])*rot(k)[n,d] (quadrature weight
    w folded into the host-side phi_tok copy), then the tiny
    kv[d,e] = sum_din MT[din,d]*WvT[din,e]; all matmuls fp16; trig
    tables computed on host in both layouts.  fp8/DoubleRow was
    evaluated and rejected: e4m3 on even one GEMM costs 2e-2 rel
    error, the whole accuracy budget.
  - No GpSimd elementwise anywhere: v8's GpSimd brot muls stretched
    every concurrent DVE op 2-4x and degraded the PE stream itself
    (removing them took the per-MM issue gap from 259ns to the
    warm-clock 216ns).
  - Phase-A rotary quad-packed: k16/krot tiles are [P, 2, 1024]
    (a-halves | b-halves of 4 chunks), 6 DVE ops of [128,1024] per
    quad instead of 24 of [128,256]; the k16 PSUM evacuation scatters
    a|b via a strided out AP and the MT matmuls read the quad through
    a [128,2,256] strided AP.  MT matmuls lag the k projections by
    TWO quads: the k16+krot chain (~4.9us serial) is longer than one
    quad's 16-MM cover.  Phase-B rotary ops are [128,2048] via
    [P,4,QT] qd/qr tiles.
  - DMA: the three HWDGE queues (sync/scalar/gpsimd) share ~358 GB/s
    aggregate (ramping from ~60 GB/s over the first ~10us) and drain
    FIFO, so transfers are scheduled by deadline: phase-A-critical
    tensors stream on the Sync queue in exact consumption order;
    phitok uses a rolling 2-block pool on the gpsimd queue paced by
    WAW pre-writes + pool WAR deps; the phase-B cq/sq tables are
    paced by WAW pre-writes keyed to mid-phase quads (bare queue
    position gets hoisted by the scheduler).  Host ships phiT/wq in
    block-major layouts so every critical DMA moves >=2KB contiguous
    runs (small-run transfers get starved under queue competition).
  - PE warm-up: dummy matmuls on a memset tile during the startup DMA
    window keep the HAM activity monitor from dropping the first real
    matmuls to half clock.
  - A/B boundary: the MT evacuation is emitted first (leads the
    scalar queue), b1(1) runs from a stack-scoped PSUM pool to give
    PE cover, and the kv matmuls use their own 2-buffer pool on the
    freed mt banks (accumulating into the mt_ps tile directly would
    serialize each kv group behind the previous group's evacuation).
  - qd evacuations all on Scalar (Vector carries only brot + half the
    b2 evacuations); b2 evacuations split Scalar/Vector so o-banks
    free in one copy latency; qr pool has 3 bufs so brot(q+2) isn't
    WAR-blocked on b2(q)'s reads of the recycled tile; final-quarter
    DMAs drain in halves on two queues (half1 issues on the idle
    gpsimd queue so the scalar queue is clear for the tail-critical
    copies), with the very last block as four 64KB pieces so the
    kernel-end DMA wait is one small transfer.

Note bq/bk/bv are all-zero by construction in this problem's input spec
(fill: zeros), so the kernel does not add them.
"""

import numpy as np
from math import sqrt

import concourse.bacc as bacc
import concourse.mybir as mybir
import concourse.tile as tile
from concourse.bass_utils import run_bass_kernel_spmd

B, N, D = 8, 4096, 512
NH = D // 2          # 256 rotary pairs
P = 128              # SBUF partitions
KC = D // P          # 4 contraction chunks of 128
NC128 = N // P       # 32 token chunks of 128 (phase A)
NQUAD = NC128 // 4   # 8 quads of 4 chunks
TB = 512             # token block (free dim of q/out matmuls)
NQ = 4               # phase-B quarters
QT = N // NQ         # 1024 tokens per quarter
F32 = mybir.dt.float32
F16 = mybir.dt.float16
COPY = mybir.ActivationFunctionType.Copy

_CACHE = {}


def _emit(nc, tc, tens):
    """Emit the per-core Tile program. tens: dict of DRAM APs."""
    from contextlib import ExitStack

    mm = nc.tensor.matmul
    ctx = tc._emit_ctx  # closed before TileContext exits
    phiT, phiT0, phi_tok = tens["phiT"], tens["phiT0"], tens["phi_tok"]
    wq, wk, wv = tens["wq"], tens["wk"], tens["wv"]
    ck_h, sk_h, cqT, sqT = tens["ck_h"], tens["sk_h"], tens["cqT"], tens["sqT"]
    outT = tens["outT"]

    # ---------------- persistent SBUF tiles ----------------
    const = ctx.enter_context(tc.tile_pool(name="const", bufs=1))
    wq_sb = const.tile([P, 2, KC, 2 * P], F16, name="wq_sb", tag="wq_sb")
    phiT_sb = const.tile([P, NQ, KC, QT], F16, name="phiT_sb", tag="phiT_sb")
    wk_sb = const.tile([P, KC, D], F16, name="wk_sb", tag="wk_sb")
    wv_sb = const.tile([P, KC, D], F16, name="wv_sb", tag="wv_sb")
    cq_sb = const.tile([P, 2, N], F16, name="cq_sb", tag="cq_sb")
    sq_sb = const.tile([P, 2, N], F16, name="sq_sb", tag="sq_sb")
    kv_sb = const.tile([P, KC, D], F16, name="kv_sb", tag="kv_sb")

    # PE warm-up source; memset is the gpsimd queue's first instruction
    warm_pool = ctx.enter_context(tc.tile_pool(name="warm", bufs=1))
    wsrc = warm_pool.tile([P, TB], F16, name="wsrc", tag="wsrc")
    nc.gpsimd.memset(wsrc[:], 0)

    # Startup critical path: the pre-phase needs wq halves + phiT
    # block 0; spread across the three HWDGE queues, one transfer
    # deep, before anything else is enqueued.
    nc.scalar.dma_start(out=wq_sb[:, 0], in_=wq[:, 0])
    nc.sync.dma_start(out=phiT_sb[:, 0, :, 0:TB], in_=phiT0[:, 0])
    nc.gpsimd.dma_start(out=phiT_sb[:, 0, 2:4, TB:QT], in_=phiT0[:, 1, 2:4])
    nc.scalar.dma_start(out=wq_sb[:, 1], in_=wq[:, 1])
    nc.sync.dma_start(out=phiT_sb[:, 0, 0:2, TB:QT], in_=phiT0[:, 1, 0:2])
    nc.sync.dma_start(out=wk_sb[:], in_=wk[:])

    # qd tiles are produced by b1 and consumed by brot one-or-more
    # pipeline stages later (qd0 spans pre-phase to phase B): 3 bufs.
    qd_pool = ctx.enter_context(tc.tile_pool(name="qd", bufs=3))

    def b1(q4, q_pool, pre=False):
        """q projection for quarter q4, d-major: qd [P, KC, QT] fp16.

        qp is a 2-bank PSUM tile (both TB halves) so each dh group
        needs ONE merged scalar evacuation.  pre=True (quarter 0,
        during the input DMA): all qp0-half matmuls run before the
        qp1 halves so the first 16 matmuls only need phiT cols 0:512."""
        qd = qd_pool.tile([P, KC, QT], F16, name="qd", tag="qd")

        def wql(dh):
            return wq_sb[:, dh // 2, :, (dh % 2) * P:(dh % 2 + 1) * P]

        def half_mms(qp, dh, h):
            cols = slice(0, TB) if h == 0 else slice(TB, QT)
            for kc in range(KC):
                mm(qp[:], wql(dh)[:, kc], phiT_sb[:, q4, kc, cols],
                   start=(kc == 0), stop=(kc == KC - 1))

        def evac(qp, dh, h):
            # Phase B: all on Scalar (Vector carries only brot, so the
            # brot->b2 latency chain at the pipeline tail stays short).
            # Pre-phase: split Scalar/Vector -- Vector is idle there,
            # and phase A's first k_ps matmuls reuse these PSUM banks,
            # so the last evacuations gate the phase-A start.
            if h == 1 and pre:
                nc.vector.tensor_copy(qd[:, dh, TB:QT], qp[:])
            elif h == 0:
                nc.scalar.copy(qd[:, dh, 0:TB], qp[:])
            else:
                nc.scalar.copy(qd[:, dh, TB:QT], qp[:])

        if pre:
            # 12 matmuls against phiT cols 0:512 before the first one
            # that needs cols 512:1024 (which is still streaming in)
            qps = {}
            for dh in range(3):
                qps[dh] = q_pool.tile([P, TB], F32, name="qp", tag="qp")
                half_mms(qps[dh], dh, 0)
            for dh in range(KC):
                if dh == 3:
                    qps[3] = q_pool.tile([P, TB], F32, name="qp", tag="qp")
                    half_mms(qps[3], 3, 0)
                evac(qps[dh], dh, 0)
                qp1 = q_pool.tile([P, TB], F32, name="qp", tag="qp")
                half_mms(qp1, dh, 1)
                evac(qp1, dh, 1)
            return qd

        for dh in range(KC):
            qp0 = q_pool.tile([P, TB], F32, name="qp", tag="qp")
            qp1 = q_pool.tile([P, TB], F32, name="qp", tag="qp")
            for kc in range(KC):
                mm(qp0[:], wql(dh)[:, kc], phiT_sb[:, q4, kc, 0:TB],
                   start=(kc == 0), stop=(kc == KC - 1))
                mm(qp1[:], wql(dh)[:, kc], phiT_sb[:, q4, kc, TB:QT],
                   start=(kc == 0), stop=(kc == KC - 1))
            evac(qp0, dh, 0)
            evac(qp1, dh, 1)
        return qd

    def brot(q4, qd, qr_pool, qm_pool):
        """rotary on q: 6 DVE ops of [128, 2048] (both halves merged)."""
        qr = qr_pool.tile([P, KC, QT], F16, name="qr", tag="qr")
        a = qd[:, 0:2, :]
        bb = qd[:, 2:4, :]
        c_ = cq_sb[:, :, q4 * QT:(q4 + 1) * QT]
        s_ = sq_sb[:, :, q4 * QT:(q4 + 1) * QT]
        w1 = qm_pool.tile([P, 2, QT], F16, name="w1", tag="wa")
        nc.vector.tensor_mul(w1[:], a, c_)
        w2 = qm_pool.tile([P, 2, QT], F16, name="w2", tag="wb")
        nc.vector.tensor_mul(w2[:], bb, s_)
        nc.vector.tensor_sub(qr[:, 0:2, :], w1[:], w2[:])
        w3 = qm_pool.tile([P, 2, QT], F16, name="w3", tag="wa")
        nc.vector.tensor_mul(w3[:], a, s_)
        w4 = qm_pool.tile([P, 2, QT], F16, name="w4", tag="wb")
        nc.vector.tensor_mul(w4[:], bb, c_)
        nc.vector.tensor_add(qr[:, 2:4, :], w3[:], w4[:])
        return qr

    def b2(q4, qr, o_pool, oq_pool, last=False):
        """outT[e, tok] for quarter q4, kv-chunk-stationary matmuls."""
        t0 = q4 * QT
        for ec in range(KC):
            o0 = o_pool.tile([P, TB], F32, name="o0", tag="o")
            o1 = o_pool.tile([P, TB], F32, name="o1", tag="o")
            for dc in range(KC):
                lhs = kv_sb[:, dc, ec * P:(ec + 1) * P]
                mm(o0[:], lhs, qr[:, dc, 0:TB],
                   start=(dc == 0), stop=(dc == KC - 1))
                mm(o1[:], lhs, qr[:, dc, TB:QT],
                   start=(dc == 0), stop=(dc == KC - 1))
            oq = oq_pool.tile([P, QT], F16, name="oq", tag="oq")
            if last and ec == KC - 1:
                # the very last output block: evacuate in four [128,256]
                # pieces, alternating engines, each DMA'd immediately on
                # its own queue so the kernel tail is one small piece
                # deep instead of a serialized [128,1024] chain
                engs = [(nc.scalar.copy, nc.sync),
                        (lambda o, i: nc.vector.tensor_copy(o, i), nc.scalar),
                        (nc.scalar.copy, nc.sync),
                        (lambda o, i: nc.vector.tensor_copy(o, i), nc.scalar)]
                for pc in range(4):
                    cols = slice(pc * NH, (pc + 1) * NH)
                    src = o0 if pc < 2 else o1
                    scols = slice((pc % 2) * NH, (pc % 2 + 1) * NH)
                    cp, q = engs[pc]
                    cp(oq[:, cols], src[:, scols])
                    q.dma_start(
                        out=outT[ec * P:(ec + 1) * P, t0 + pc * NH:t0 + (pc + 1) * NH],
                        in_=oq[:, cols])
                return
            # split evacuation across engines so the PSUM banks free in
            # one copy-latency (gpsimd can't read PSUM); for the
            # second-to-last block both go on vector so the scalar
            # queue is clear for the final pieces
            if last and ec == KC - 2:
                nc.vector.tensor_copy(oq[:, 0:TB], o0[:])
                nc.vector.tensor_copy(oq[:, TB:QT], o1[:])
            else:
                nc.scalar.copy(oq[:, 0:TB], o0[:])
                nc.vector.tensor_copy(oq[:, TB:QT], o1[:])
            if last:
                if ec == KC - 1:
                    # the very last block drains as four 64KB pieces on
                    # two queues: the kernel-end DMA wait shrinks to one
                    # small transfer instead of a 128KB one
                    qs = [nc.sync, nc.scalar, nc.scalar, nc.sync]
                    for pc in range(4):
                        qs[pc].dma_start(
                            out=outT[ec * P:(ec + 1) * P,
                                     t0 + pc * NH:t0 + (pc + 1) * NH],
                            in_=oq[:, pc * NH:(pc + 1) * NH])
                else:
                    # halves on different queues so the tail drains in
                    # parallel; half1 issues ride the idle gpsimd queue
                    # so the scalar queue is clear for the final
                    # block's evacuation copies
                    nc.sync.dma_start(
                        out=outT[ec * P:(ec + 1) * P, t0:t0 + TB], in_=oq[:, 0:TB])
                    nc.gpsimd.dma_start(
                        out=outT[ec * P:(ec + 1) * P, t0 + TB:t0 + QT], in_=oq[:, TB:QT])
            else:
                nc.sync.dma_start(
                    out=outT[ec * P:(ec + 1) * P, t0:t0 + QT], in_=oq[:])

    # ============ pre-phase: PE warm-up + b1(0) during the input DMA ============
    with ExitStack() as pctx:
        wps_pool = pctx.enter_context(tc.tile_pool(name="wps", bufs=1, space="PSUM"))
        wps = wps_pool.tile([P, TB], F32, name="wps", tag="wps")
        # dummy PE work fills the HAM activity window while the first
        # real operands stream in, so real MMs start at full clock
        for _ in range(10):
            mm(wps[:], wsrc[:, 0:P], wsrc[:], start=True, stop=True)
        for _ in range(2):
            mm(wps[:, 0:P], wsrc[:, 0:P], wsrc[:, 0:P], start=True, stop=True)

        qpre_pool = pctx.enter_context(tc.tile_pool(name="qpre", bufs=4, space="PSUM"))
        qd0 = b1(0, qpre_pool, pre=True)

    # ================ phase A: MT = phi_tok^T rot(w*k) ================
    with ExitStack() as actx:
        aconst = actx.enter_context(tc.tile_pool(name="aconst", bufs=1))
        mt_sb = aconst.tile([P, KC, D], F16, name="mt_sb", tag="mt_sb")

        mtctx = ExitStack()
        mt_pool = mtctx.enter_context(tc.tile_pool(name="mt_ps", bufs=1, space="PSUM"))
        mt_ps = mt_pool.tile([P, KC, D], F32, name="mt_ps", tag="mt_ps")

        with ExitStack() as kctx:
            kconst = kctx.enter_context(tc.tile_pool(name="kconst", bufs=1))
            ck_sb = kconst.tile([P, NC128 * NH], F16, name="ck_sb", tag="ck_sb")
            sk_sb = kconst.tile([P, NC128 * NH], F16, name="sk_sb", tag="sk_sb")
            ptok_pool = kctx.enter_context(tc.tile_pool(name="ptok", bufs=2))

            # Sync queue carries every phase-A-deadline transfer in
            # exact consumption order (self-pacing FIFO).
            for blk in range(4):
                cols = slice(blk * 2048, (blk + 1) * 2048)
                nc.sync.dma_start(out=ck_sb[:, cols], in_=ck_h[:, cols])
                nc.sync.dma_start(out=sk_sb[:, cols], in_=sk_h[:, cols])
                if blk in (0, 1, 2):
                    nc.sync.dma_start(out=phiT_sb[:, 1 + blk], in_=phiT[:, 1 + blk])
            nc.sync.dma_start(out=wv_sb[:], in_=wv[:])

            # phitok: rolling 2-block pool on the gpsimd queue.  A DMA
            # is only reliably delayed by a DATA dependency (the
            # scheduler hoists independent issues), so blocks 0-1 get a
            # tiny WAW pre-write into their destination keyed to the
            # pre-phase output; blocks 2-3 self-pace on the pool's WAR
            # dependencies.  Keeps gpsimd off the early-startup
            # bandwidth that wq/phiT0 need.
            ptok = []
            for blk in range(4):
                t_ = ptok_pool.tile([P, 8 * D], F16, name="ptok", tag="ptok")
                if blk == 0:
                    nc.gpsimd.tensor_copy(t_[:, 0:1], qd0[:, 0, 0:1])
                elif blk == 1:
                    nc.gpsimd.tensor_copy(t_[:, 0:1], qd0[:, 3, 0:1])
                tcols = slice(blk * 8 * D, (blk + 1) * 8 * D)
                nc.gpsimd.dma_start(out=t_[:], in_=phi_tok[:, tcols])
                ptok.append(t_)

            k_pool = kctx.enter_context(tc.tile_pool(name="k_ps", bufs=3, space="PSUM"))
            k16_pool = kctx.enter_context(tc.tile_pool(name="k16p", bufs=2))
            krot_pool = kctx.enter_context(tc.tile_pool(name="krotp", bufs=3))
            tmp_pool = kctx.enter_context(tc.tile_pool(name="tmpA", bufs=1))

            # MT matmuls of quad t run TWO quads behind the k
            # projections: the krot chain (k16 cast + 6 serial DVE ops
            # ~4.9us) is longer than one quad's 16-MM cover (~3.5us).
            krotqs = {}

            def mt_mms(t):
                krotq = krotqs.pop(t)
                pt = ptok[t // 2]
                for j in range(4):
                    c = 4 * t + j
                    cc = c % 8
                    for dinc in range(KC):
                        mm(mt_ps[:, dinc, :],
                           pt[:, cc * D + dinc * P: cc * D + (dinc + 1) * P],
                           krotq[:, :, j * NH:(j + 1) * NH],
                           start=(c == 0), stop=(c == NC128 - 1))

            for t in range(NQUAD):
                # k projection (w folded into the PSUM evacuation
                # scale); evacuation scatters chunk c into the packed
                # [a|b] quad layout.
                # (the quadrature weight w is folded into phi_tok on
                # the host, so the evacuation is a plain cast)
                k16q = k16_pool.tile([P, 2, 4 * NH], F16, name="k16q", tag="k16q")
                for j in range(4):
                    c = 4 * t + j
                    blk, cc = c // 8, c % 8
                    tok = slice(cc * P, (cc + 1) * P)
                    k_ps = k_pool.tile([P, D], F32, name="k_ps", tag="k_ps")
                    for kc in range(KC):
                        mm(k_ps[:], phiT_sb[:, blk, kc, tok], wk_sb[:, kc, :],
                           start=(kc == 0), stop=(kc == KC - 1))
                    nc.scalar.copy(k16q[:, :, j * NH:(j + 1) * NH], k_ps[:])

                # k-rotary for the whole quad: 6 DVE ops of [128, 1024]
                krotq = krot_pool.tile([P, 2, 4 * NH], F16, name="krotq", tag="krotq")
                A = k16q[:, 0, :]
                Bb = k16q[:, 1, :]
                CK = ck_sb[:, t * 4 * NH:(t + 1) * 4 * NH]
                SK = sk_sb[:, t * 4 * NH:(t + 1) * 4 * NH]
                m1 = tmp_pool.tile([P, 4 * NH], F16, name="m1", tag="ma")
                nc.vector.tensor_mul(m1[:], A, CK)
                m2 = tmp_pool.tile([P, 4 * NH], F16, name="m2", tag="mb")
                nc.vector.tensor_mul(m2[:], Bb, SK)
                nc.vector.tensor_sub(krotq[:, 0, :], m1[:], m2[:])
                m3 = tmp_pool.tile([P, 4 * NH], F16, name="m3", tag="ma")
                nc.vector.tensor_mul(m3[:], A, SK)
                m4 = tmp_pool.tile([P, 4 * NH], F16, name="m4", tag="mb")
                nc.vector.tensor_mul(m4[:], Bb, CK)
                nc.vector.tensor_add(krotq[:, 1, :], m3[:], m4[:])
                krotqs[t] = krotq

                # one phase-B q-trig DMA per mid-phase quad, paced by a
                # WAW pre-write keyed to this quad's k16 (a bare queue
                # position would get hoisted by the scheduler)
                if 2 <= t < 6:
                    h = (t - 2) % 2
                    dst, src = (cq_sb, cqT) if t < 4 else (sq_sb, sqT)
                    nc.scalar.copy(dst[:, h, 0:1], k16q[:, 0, 0:1])
                    nc.scalar.dma_start(out=dst[:, h, :], in_=src[h * P:(h + 1) * P, :])

                if t >= 2:
                    mt_mms(t - 2)
            mt_mms(NQUAD - 2)
            mt_mms(NQUAD - 1)

        # ========= A/B boundary (kv computation behind b1(1)) =========
        # b1(1) is emitted first, from a temporary PSUM pool (stack-
        # ordered inside mt's lifetime), so its 32 matmuls cover the MT
        # evacuation + kv chain on the PE stream.
        with ExitStack() as kvctx:
            # MT evacuation emitted FIRST so it leads the scalar queue
            # (its deps complete before b1(1)'s); b1(1)'s 32 matmuls
            # then cover it on the PE stream before the kv matmuls.
            for dinc in range(KC):
                nc.scalar.copy(mt_sb[:, dinc, :], mt_ps[:, dinc, :])

            qb_pool = kvctx.enter_context(
                tc.tile_pool(name="qb_ps", bufs=4, space="PSUM"))
            qd1 = b1(1, qb_pool)
        mtctx.close()

        # kv[d,e] = sum_din MT[din,d] * WvT[din,e]  (tiny 512^3 matmul)
        # in its own pool on the freed mt banks: accumulating into the
        # mt_ps tile directly would serialize each kv group behind the
        # previous group's evacuation (intra-tile WAR).
        with ExitStack() as kv2ctx:
            kv_pool = kv2ctx.enter_context(
                tc.tile_pool(name="kv_ps", bufs=2, space="PSUM"))
            for dco in range(KC):
                kvp = kv_pool.tile([P, D], F32, name="kvp", tag="kvp")
                for dinc in range(KC):
                    mm(kvp[:], mt_sb[:, dinc, dco * P:(dco + 1) * P], wv_sb[:, dinc, :],
                       start=(dinc == 0), stop=(dinc == KC - 1))
                nc.scalar.copy(kv_sb[:, dco, :], kvp[:])

        # ================ phase B: outT = kv^T rot(q)^T ================
        q_pool = actx.enter_context(tc.tile_pool(name="q_ps", bufs=3, space="PSUM"))
        qr_pool = actx.enter_context(tc.tile_pool(name="qr", bufs=2))
        qm_pool = actx.enter_context(tc.tile_pool(name="qm", bufs=2))
        oq_pool = actx.enter_context(tc.tile_pool(name="oq", bufs=2))
        o_pool = actx.enter_context(tc.tile_pool(name="o_ps", bufs=5, space="PSUM"))

        # software pipeline: b1 runs ahead so every brot has PE cover
        qr0 = brot(0, qd0, qr_pool, qm_pool)
        qd2 = b1(2, q_pool)
        qr1 = brot(1, qd1, qr_pool, qm_pool)
        b2(0, qr0, o_pool, oq_pool)
        qd3 = b1(3, q_pool)
        qr2 = brot(2, qd2, qr_pool, qm_pool)
        b2(1, qr1, o_pool, oq_pool)
        qr3 = brot(3, qd3, qr_pool, qm_pool)
        b2(2, qr2, o_pool, oq_pool)
        b2(3, qr3, o_pool, oq_pool, last=True)


def _build(reps=1):
    """Build + schedule + compile the single-core program (shared SPMD)."""
    if reps in _CACHE:
        return _CACHE[reps]
    from contextlib import ExitStack

    nc = bacc.Bacc("TRN2", target_bir_lowering=False, debug=False,
                   enable_asserts=False, num_devices=B)
    tens = {
        "phiT": nc.dram_tensor("phiT", [P, NQ, KC, QT], F16, kind="ExternalInput").ap(),
        "phiT0": nc.dram_tensor("phiT0", [P, 2, KC, TB], F16, kind="ExternalInput").ap(),
        "phi_tok": nc.dram_tensor("phi_tok", [P, NC128 * D], F16, kind="ExternalInput").ap(),
        "wq": nc.dram_tensor("wq", [P, 2, KC, 2 * P], F16, kind="ExternalInput").ap(),
        "wk": nc.dram_tensor("wk", [P, KC, D], F16, kind="ExternalInput").ap(),
        "wv": nc.dram_tensor("wv", [P, KC, D], F16, kind="ExternalInput").ap(),
        "ck_h": nc.dram_tensor("ck_h", [P, NC128 * NH], F16, kind="ExternalInput").ap(),
        "sk_h": nc.dram_tensor("sk_h", [P, NC128 * NH], F16, kind="ExternalInput").ap(),
        "cqT": nc.dram_tensor("cqT", [2 * P, N], F16, kind="ExternalInput").ap(),
        "sqT": nc.dram_tensor("sqT", [2 * P, N], F16, kind="ExternalInput").ap(),
        "outT": nc.dram_tensor("outT", [D, N], F16, kind="ExternalOutput").ap(),
    }
    with tile.TileContext(nc) as tc:
        for _ in range(reps):
            with ExitStack() as ctx:
                tc._emit_ctx = ctx
                _emit(nc, tc, tens)
    nc.compile()
    _CACHE[reps] = nc
    return nc


def _in_maps(phi, coords, weights, Wq, Wk, Wv, Wrot):
    """Host-side layout prep + per-core input maps (batch b -> core b)."""
    phi = np.asarray(phi, dtype=np.float32)
    coords = np.asarray(coords, dtype=np.float32)
    weights = np.asarray(weights, dtype=np.float32)
    phiT2 = np.ascontiguousarray(phi.transpose(0, 2, 1)).astype(np.float16)  # [B,D,N]
    # block-major phiT: [P, NQ, KC, QT]; block 0 also as halves [P,2,KC,TB]
    phiT = np.ascontiguousarray(
        phiT2.reshape(B, KC, P, NQ, QT).transpose(0, 2, 3, 1, 4))
    phiT0 = np.ascontiguousarray(
        phiT2[:, :, 0:QT].reshape(B, KC, P, 2, TB).transpose(0, 2, 3, 1, 4))
    # quadrature weight w folded into the phase-A phi copy on the host:
    # kv = sum_n (w_n phi[n,:]) (x) rot(k)[n,:]
    phiw16 = (weights[..., None] * phi).astype(np.float16)
    phi_tok = np.ascontiguousarray(
        phiw16.reshape(B, NC128, P, D).transpose(0, 2, 1, 3).reshape(B, P, NC128 * D))
    wqT = np.ascontiguousarray(
        np.asarray(Wq, np.float32).T / sqrt(D)).astype(np.float16)
    wq = np.ascontiguousarray(
        wqT.reshape(KC, P, 2, 2 * P).transpose(1, 2, 0, 3))        # [P,2,KC,256]
    wk = np.ascontiguousarray(
        np.asarray(Wk, np.float32).T.astype(np.float16).reshape(KC, P, D).transpose(1, 0, 2))
    wv = np.ascontiguousarray(
        np.asarray(Wv, np.float32).T.astype(np.float16).reshape(KC, P, D).transpose(1, 0, 2))

    # host trig: phases [B, N, NH] -> cos/sin in both layouts
    ph = np.einsum('bnc,dc->bnd', coords, np.asarray(Wrot, np.float32))
    cos = np.cos(ph).astype(np.float16)    # [B, N, NH]
    sin = np.sin(ph).astype(np.float16)
    # token-major: [P, NC128*NH], chunk c at cols c*NH, row p = token c*P+p
    ck_h = np.ascontiguousarray(
        cos.reshape(B, NC128, P, NH).transpose(0, 2, 1, 3).reshape(B, P, NC128 * NH))
    sk_h = np.ascontiguousarray(
        sin.reshape(B, NC128, P, NH).transpose(0, 2, 1, 3).reshape(B, P, NC128 * NH))
    # d-major: [2P, N] rows = pair index, cols = token
    cqT = np.ascontiguousarray(cos.transpose(0, 2, 1))             # [B, NH, N]
    sqT = np.ascontiguousarray(sin.transpose(0, 2, 1))
    return [
        {"phiT": phiT[b], "phiT0": phiT0[b], "phi_tok": phi_tok[b],
         "wq": wq, "wk": wk, "wv": wv,
         "ck_h": ck_h[b], "sk_h": sk_h[b], "cqT": cqT[b], "sqT": sqT[b]}
        for b in range(B)
    ]


def kernel(phi, coords, weights, Wq, bq, Wk, bk, Wv, bv, Wrot, **run_kwargs):
    """Full inputs in, full output out. bq/bk/bv are zeros by input spec."""
    nc = _build(1)
    in_maps = _in_maps(phi, coords, weights, Wq, Wk, Wv, Wrot)
    res = run_bass_kernel_spmd(nc, in_maps, list(range(B)), **run_kwargs)
    out = np.stack([res.results[b]["outT"].astype(np.float32).T
                    for b in range(B)])
    out = np.ascontiguousarray(out)
    if run_kwargs:
        kernel.last_result = res
    return out
